# revision 28
# baseline (speedup 1.0000x reference)
"""3-layer GCN (GCNConv + BN + ReLU x2 + GCNConv) on 8 Trainium2 NeuronCores.

Strategy (dst-partitioned graph parallelism):
  - Nodes are split across 8 cores (6250 each, padded to 6272 = 49*128).
  - Within a core, nodes are reordered by (n_low_edges, n_high_edges) so each
    128-node output tile has a near-uniform per-node incident-edge count; edge
    slots are padded per-tile to the tile max -> rectangular gather layouts.
  - Per layer: dense transform W on the PE (feature-major, weights stationary),
    scale columns by dinv and cast to fp16, transpose back to node-major rows,
    write a 256B-strided fp16 node-feature table to DRAM, AllGather it,
    then bulk dma_gather (int16 idx; payload C_out*2 bytes, 256B row stride)
    of every edge's source row, and a strided DVE tensor_reduce per output
    tile.  BN+ReLU is a per-feature affine fused into one ACT op per tile.
  - norm(e) = dinv[src]*dinv[dst]: dinv[src] is folded into the table rows,
    dinv[dst] is applied to the aggregated tile.
  - int16 gather indices cover at most 32767 rows, so edges are split by
    source table row at ROW_SPLIT = 5*6272; low/high sides gather from offset
    views of the same table.  The (n_low, n_high) node sort keeps the
    per-tile padding of both sides small.

Everything data-dependent (degrees, permutations, slot layouts, indices) is
precomputed on the host in numpy; the device program is static SPMD (all 8
cores run the same NEFF with different input arrays).
"""
import sys

sys.path.insert(0, "/opt/trn_rl_repo")

import numpy as np

N_NODES = 50000
IN_C, HID_C, OUT_C = 128, 64, 40
BN_EPS = 1e-5
N_CORES = 8
NPC = N_NODES // N_CORES          # 6250
TILES = 49
NPAD = TILES * 128                # 6272
TABLE_ROWS = N_CORES * NPAD       # 50176
OSPLIT = 5 * NPC                  # original-id split (31250)
ROW_SPLIT = 5 * NPAD              # table-row split (31360) < 32768
SLOT_E = 64                       # fp16 elems per table row
SLOT_B = SLOT_E * 2               # bytes per SBUF table rank-slot
RANKS = TABLE_ROWS // 128         # 392 SBUF table slots per partition
BATCH_SLOTS = 40                  # gather slots per dma_gather instruction
N_QUEUES = 4

_COMPILED = {}


# ----------------------------------------------------------------- host plan
def _build_plan(edge_index):
    ei = np.asarray(edge_index)
    loops = np.arange(N_NODES, dtype=np.int64)
    src = np.concatenate([ei[0].astype(np.int64), loops])
    dst = np.concatenate([ei[1].astype(np.int64), loops])
    deg = np.bincount(dst, minlength=N_NODES).astype(np.float64)
    dinv = (1.0 / np.sqrt(deg)).astype(np.float32)

    is_low = src < OSPLIT
    core_of_dst = dst // NPC

    # per-core node stats and ordering (local ids 0..6271; >=6250 are dummies)
    orders = []           # per core: local real/dummy id at each pi position
    nl_ord = np.zeros((N_CORES, NPAD), np.int64)
    nh_ord = np.zeros((N_CORES, NPAD), np.int64)
    pos_of = np.empty(N_NODES, np.int64)  # pi position of each original node
    for k in range(N_CORES):
        m = core_of_dst == k
        dl = dst[m] - k * NPC
        nl = np.bincount(dl[is_low[m]], minlength=NPAD)
        nh = np.bincount(dl[~is_low[m]], minlength=NPAD)
        order = np.lexsort((nh, nl))  # ascending; dummies (0,0) first
        # boustrophedon on the secondary key: within each nl-group alternate
        # nh direction so tiles straddling group boundaries stay homogeneous
        vals = nl[order]
        segs, flip, i = [], False, 0
        while i < len(order):
            j = i
            while j < len(order) and vals[j] == vals[i]:
                j += 1
            segs.append(order[i:j][::-1] if flip else order[i:j])
            flip = not flip
            i = j
        order = np.concatenate(segs)
        # local refinement: within adjacent tile pairs whose nl spread is <=1,
        # re-sort by nh to shave the per-tile nh max
        for _ in range(2):
            for t in range(0, TILES - 1):
                seg = order[t * 128 : (t + 2) * 128]
                a = nl[seg]
                if a.max() - a.min() <= 1:
                    order[t * 128 : (t + 2) * 128] = seg[
                        np.argsort(nh[seg], kind="stable")]
        orders.append(order)
        nl_ord[k] = nl[order]
        nh_ord[k] = nh[order]
        inv = np.empty(NPAD, np.int64)
        inv[order] = np.arange(NPAD)
        pos_of[k * NPC : (k + 1) * NPC] = inv[:NPC]

    row_of = (np.arange(N_NODES) // NPC) * NPAD + pos_of  # table row per node

    # common per-tile slot counts
    kA = nl_ord.reshape(N_CORES, TILES, 128).max(axis=2).max(axis=0)
    kB = nh_ord.reshape(N_CORES, TILES, 128).max(axis=2).max(axis=0)
    baseA = np.concatenate([[0], np.cumsum(kA)[:-1]])
    baseB = np.concatenate([[0], np.cumsum(kB)[:-1]])
    slotsA, slotsB = int(kA.sum()), int(kB.sum())

    Z_LO = 0                       # core 0 dummy row (always zero)
    Z_HI = 7 * NPAD - ROW_SPLIT    # core 7 dummy row, rebased

    def side_vals(k, low):
        m = (core_of_dst == k) & (is_low if low else ~is_low)
        sp = pos_of[dst[m]]  # pi position of dst within its core
        rows = row_of[src[m]]
        o = np.argsort(sp, kind="stable")
        sp, rows = sp[o], rows[o]
        counts = np.bincount(sp, minlength=NPAD)
        starts = np.concatenate([[0], np.cumsum(counts)[:-1]])
        occ = np.arange(len(sp)) - starts[sp]
        t, p = sp // 128, sp % 128
        base = baseA if low else baseB
        nslots = slotsA if low else slotsB
        vals = np.full(nslots * 128, Z_LO if low else Z_HI, np.int64)
        pos = (base[t] + occ) * 128 + p
        vals[pos] = rows if low else rows - ROW_SPLIT
        return vals

    def wrap16(v):
        n = v.shape[0]
        cols = n // 16
        t = v.astype(np.int16).reshape(cols, 16).T
        return np.tile(t, (8, 1))

    idxA = np.stack([wrap16(side_vals(k, True)) for k in range(N_CORES)])
    idxB = np.stack([wrap16(side_vals(k, False)) for k in range(N_CORES)])

    # gather batches: tile-aligned runs with <= BATCH_SLOTS slots
    def make_batches(kvec):
        batches = []  # (tile0, tile1, slot0, nslots)
        t0, s0, acc = 0, 0, 0
        for t in range(TILES):
            if acc + kvec[t] > BATCH_SLOTS and acc > 0:
                batches.append((t0, t, s0, acc))
                t0, s0, acc = t, s0 + acc, 0
            acc += int(kvec[t])
        batches.append((t0, TILES, s0, acc))
        return batches

    batchesA, batchesB = make_batches(kA), make_batches(kB)

    dinv_ord = np.zeros((N_CORES, NPAD), np.float32)
    for k in range(N_CORES):
        o = orders[k]
        real = o < NPC
        dinv_ord[k][real] = dinv[k * NPC + o[real]]

    return dict(
        orders=orders, dinv_ord=dinv_ord, idxA=idxA, idxB=idxB,
        kA=kA, kB=kB, baseA=baseA, baseB=baseB,
        slotsA=slotsA, slotsB=slotsB,
        batchesA=batchesA, batchesB=batchesB,
    )


# ------------------------------------------------------------- device build
def _build_bass(plan):
    import concourse.bacc as bacc
    import concourse.bass as bass
    import concourse.tile as tile
    from concourse import ap_utils, mybir
    from concourse.masks import make_identity

    kA, kB = plan["kA"], plan["kB"]
    baseA, baseB = plan["baseA"], plan["baseB"]
    slotsA, slotsB = plan["slotsA"], plan["slotsB"]
    batchesA, batchesB = plan["batchesA"], plan["batchesB"]
    CA, CB = slotsA * 8, slotsB * 8

    fp16, f32, i16 = mybir.dt.float16, mybir.dt.float32, mybir.dt.int16

    nc = bacc.Bacc(None, target_bir_lowering=False, num_swdge_queues=N_QUEUES)

    xT = nc.declare_dram_parameter("xT", [IN_C, NPAD], fp16, isOutput=False)
    idxA_p = nc.declare_dram_parameter("idxA", [128, CA], i16, isOutput=False)
    idxB_p = nc.declare_dram_parameter("idxB", [128, CB], i16, isOutput=False)
    dinv_row_p = nc.declare_dram_parameter("dinv_row", [1, NPAD], f32, isOutput=False)
    dinv_col_p = nc.declare_dram_parameter("dinv_col", [128, TILES], f32, isOutput=False)
    W_p = [
        nc.declare_dram_parameter("W0", [IN_C, HID_C], fp16, isOutput=False),
        nc.declare_dram_parameter("W1", [HID_C, HID_C], fp16, isOutput=False),
        nc.declare_dram_parameter("W2", [HID_C, OUT_C], fp16, isOutput=False),
    ]
    A_p = [
        nc.declare_dram_parameter("A0", [HID_C, 1], f32, isOutput=False),
        nc.declare_dram_parameter("A1", [HID_C, 1], f32, isOutput=False),
    ]
    B_p = [
        nc.declare_dram_parameter("B0", [HID_C, 1], f32, isOutput=False),
        nc.declare_dram_parameter("B1", [HID_C, 1], f32, isOutput=False),
    ]
    b2_p = nc.declare_dram_parameter("b2", [1, OUT_C], f32, isOutput=False)
    out_p = nc.declare_dram_parameter("out", [NPAD, OUT_C], f32, isOutput=True)

    C_OUT = [HID_C, HID_C, OUT_C]
    C_IN = [IN_C, HID_C, HID_C]

    reg_cache = {}

    def nir(n):
        if n not in reg_cache:
            reg_cache[n] = nc.gpsimd.to_reg(n)
        return reg_cache[n]

    def gather(out_ap, in_ap, idxs_ap, num_idxs, elem_size, queue):
        # SBUF-source gather: idx -> partition idx%128, rank idx//128 at
        # 128B-per-rank free stride within in_ap's base.
        self = nc.gpsimd
        return self.add_instruction(
            mybir.InstDMAGatherAnt(
                name=nc.get_next_instruction_name(),
                ins=[self.lower_ap(in_ap), self.lower_ap(idxs_ap),
                     self.lower_val_access(nir(num_idxs))],
                outs=[self.lower_ap(out_ap)],
                transpose=False,
                num_idxs=num_idxs,
                elem_size=elem_size,
                stride_bytes_256=0,
                gen_mode=0,
                single_packet=False,
                queue_num=queue,
                sbuf_tokens_per_rank=128,
                sbuf_free_dim_per_rank=SLOT_B,
                sbuf_free_dim_pad_per_rank=0,
                sbuf_byte_offset=0,
            )
        )

    with tile.TileContext(nc) as tc:
        with (
            tc.tile_pool(name="const", bufs=1) as constp,
            tc.tile_pool(name="ht", bufs=1) as htp,
            tc.tile_pool(name="work", bufs=3) as work,
            tc.tile_pool(name="gbuf", bufs=1) as gbufp,
            tc.tile_pool(name="zs", bufs=4) as zsp,
            tc.tile_pool(name="psum", bufs=2, space="PSUM") as psum,
            tc.tile_pool(name="psum2", bufs=2, space="PSUM") as psum2,
            tc.tile_pool(name="dram", bufs=1, space="DRAM") as dram,
        ):
            ident = constp.tile([128, 128], fp16)
            make_identity(nc, ident[:])
            idxA_t = constp.tile([128, CA], i16)
            nc.sync.dma_start(out=idxA_t[:], in_=idxA_p[:])
            idxB_t = constp.tile([128, CB], i16)
            nc.sync.dma_start(out=idxB_t[:], in_=idxB_p[:])
            dinv_col = constp.tile([128, TILES], f32)
            nc.sync.dma_start(out=dinv_col[:], in_=dinv_col_p[:])
            W_t = []
            for l in range(3):
                w = constp.tile([C_IN[l], C_OUT[l]], fp16, name=f"W{l}t")
                nc.sync.dma_start(out=w[:], in_=W_p[l][:])
                W_t.append(w)
            AB_t = []
            for l in range(2):
                a = constp.tile([HID_C, 1], f32, name=f"A{l}t")
                nc.sync.dma_start(out=a[:], in_=A_p[l][:])
                b = constp.tile([HID_C, 1], f32, name=f"B{l}t")
                nc.sync.dma_start(out=b[:], in_=B_p[l][:])
                AB_t.append((a, b))
            b2_t = constp.tile([128, OUT_C], f32)
            nc.gpsimd.dma_start(
                out=b2_t[:], in_=b2_p[:].to_broadcast([128, OUT_C]))

            hT = [htp.tile([128, NPAD], fp16, name="hT0"),
                  htp.tile([HID_C, NPAD], fp16, name="hT1")]
            nc.sync.dma_start(out=hT[0][:], in_=xT[:])

            tabin = [dram.tile([128, TILES * SLOT_E], fp16, name=f"tabin{l}")
                     for l in range(3)]
            tabout = [
                dram.tile([N_CORES * 128, TILES * SLOT_E], fp16,
                          addr_space="Shared", name=f"tabout{l}")
                for l in range(3)
            ]
            tab_s = htp.tile([128, RANKS * SLOT_E], fp16, name="tab_s")

            qrr = [0]

            def next_q():
                q = qrr[0] % N_QUEUES
                qrr[0] += 1
                return q

            for l in range(3):
                cin, cout = C_IN[l], C_OUT[l]
                h_cur = hT[l % 2]
                # ---- transform + table build (feature-major) ----
                for c0 in range(0, NPAD, 512):
                    w = min(512, NPAD - c0)
                    pt = psum.tile([cout, 512], f32, tag="pt")
                    nc.tensor.matmul(pt[:, :w], W_t[l][:], h_cur[:cin, c0 : c0 + w],
                                     start=True, stop=True)
                    ts_ = work.tile([cout, 512], fp16, tag="ts")
                    nc.scalar.activation(
                        out=ts_[:, :w], in_=pt[:, :w],
                        func=mybir.ActivationFunctionType.Copy)
                    for j0 in range(0, w, 128):
                        ptr = psum2.tile([128, cout], fp16, tag="ptr")
                        nc.tensor.transpose(
                            ptr[:], ts_[:, j0 : j0 + 128], ident[:cout, :cout])
                        tb = work.tile([128, cout], fp16, tag="tb")
                        # fold dinv[src] into the table row (node-major:
                        # per-partition scale on the ACT engine)
                        nc.scalar.activation(
                            out=tb[:], in_=ptr[:],
                            func=mybir.ActivationFunctionType.Copy,
                            scale=dinv_col[:, (c0 + j0) // 128 :
                                           (c0 + j0) // 128 + 1])
                        t_i = (c0 + j0) // 128
                        nc.sync.dma_start(
                            out=tabin[l][:, t_i * SLOT_E : t_i * SLOT_E + cout],
                            in_=tb[:],
                        )
                # ---- all-gather the fp16 table ----
                nc.gpsimd.collective_compute(
                    "AllGather",
                    mybir.AluOpType.bypass,
                    replica_groups=[list(range(N_CORES))],
                    ins=[tabin[l][:]],
                    outs=[tabout[l][:]],
                )
                # ---- load the gathered table into SBUF ----
                nc.sync.dma_start(
                    out=tab_s[:].rearrange("p (k c) -> p k c", k=N_CORES),
                    in_=tabout[l][:].rearrange("(k p) c -> p k c", p=128),
                )
                # ---- gathers (A/B interleaved) + per-tile reduce/post ----
                gtiles = {}
                sides = (
                    ("A", batchesA, idxA_t, baseA, kA, tab_s[:, :]),
                    ("B", batchesB, idxB_t, baseB, kB,
                     tab_s[:, (ROW_SPLIT // 128) * SLOT_E :]),
                )

                def emit_gather(side_i, bi):
                    side, batches, idx_t, base, kvec, lo = sides[side_i]
                    t0, t1, s0, ns = batches[bi]
                    g = gbufp.tile([128, ns * cout], fp16,
                                   name=f"g{l}{side}{bi}",
                                   tag=f"g{side}{bi % 3}", bufs=2)
                    gather(
                        out_ap=g[:].rearrange("p (s f) -> p s f", f=cout),
                        in_ap=lo,
                        idxs_ap=idx_t[:, s0 * 8 : (s0 + ns) * 8],
                        num_idxs=ns * 128,
                        elem_size=cout,
                        queue=next_q(),
                    )
                    for t in range(t0, t1):
                        off = int(base[t] - s0)
                        gtiles.setdefault(t, {})[side] = (g, off, int(kvec[t]))
                    return t1

                def emit_tile(t):
                    gA, offA, ka = gtiles[t]["A"]
                    gB, offB, kb = gtiles[t]["B"]
                    z = zsp.tile([128, cout], f32, tag="z")
                    if ka > 0:
                        vA = gA[:, offA * cout : (offA + ka) * cout].rearrange(
                            "p (s f) -> p f s", s=ka)
                        nc.vector.tensor_reduce(
                            out=z[:], in_=vA, axis=mybir.AxisListType.X,
                            op=mybir.AluOpType.add)
                    if kb > 0:
                        vB = gB[:, offB * cout : (offB + kb) * cout].rearrange(
                            "p (s f) -> p f s", s=kb)
                        if ka > 0:
                            z2 = zsp.tile([128, cout], f32, tag="z2")
                            nc.vector.tensor_reduce(
                                out=z2[:], in_=vB, axis=mybir.AxisListType.X,
                                op=mybir.AluOpType.add)
                            nc.vector.tensor_add(out=z[:], in0=z[:], in1=z2[:])
                        else:
                            nc.vector.tensor_reduce(
                                out=z[:], in_=vB, axis=mybir.AxisListType.X,
                                op=mybir.AluOpType.add)
                    if l < 2:
                        zsc = zsp.tile([128, cout], fp16, tag="zsc")
                        nc.scalar.activation(
                            out=zsc[:], in_=z[:],
                            func=mybir.ActivationFunctionType.Copy,
                            scale=dinv_col[:, t : t + 1])
                        pz = psum2.tile([cout, 128], fp16, tag="pz")
                        nc.tensor.transpose(pz[:], zsc[:], ident[:])
                        a_t, bb_t = AB_t[l]
                        nc.scalar.activation(
                            out=hT[(l + 1) % 2][:cout, t * 128 : (t + 1) * 128],
                            in_=pz[:],
                            func=mybir.ActivationFunctionType.Relu,
                            bias=bb_t[:],
                            scale=a_t[:],
                        )
                    else:
                        zf = zsp.tile([128, OUT_C], f32, tag="zf")
                        nc.scalar.activation(
                            out=zf[:], in_=z[:],
                            func=mybir.ActivationFunctionType.Copy,
                            scale=dinv_col[:, t : t + 1])
                        nc.vector.tensor_add(out=zf[:], in0=zf[:], in1=b2_t[:])
                        nc.sync.dma_start(
                            out=out_p[t * 128 : (t + 1) * 128, :], in_=zf[:])

                iA = iB = 0
                covA = covB = 0
                done = 0
                while done < TILES:
                    if iA < len(batchesA):
                        covA = emit_gather(0, iA)
                        iA += 1
                    if iB < len(batchesB):
                        covB = emit_gather(1, iB)
                        iB += 1
                    if iA >= len(batchesA):
                        covA = TILES
                    if iB >= len(batchesB):
                        covB = TILES
                    for t in range(done, min(covA, covB)):
                        emit_tile(t)
                    done = max(done, min(covA, covB))

    nc.finalize()
    return nc


# ------------------------------------------------------------------- kernel
TRACE = False
LAST_EXEC_NS = None


def kernel(x, edge_index, W0, b0, gamma0, beta0, mean0, var0,
           W1, b1, gamma1, beta1, mean1, var1, W2, b2):
    global LAST_EXEC_NS
    from concourse.bass_utils import run_bass_kernel_spmd

    x = np.asarray(x, np.float32)
    plan = _build_plan(edge_index)

    key = "k"
    if key not in _COMPILED:
        _COMPILED[key] = _build_bass(plan)
    nc = _COMPILED[key]

    # fold BN (eval mode) into per-feature affines: y = z*A + B
    def fold(gamma, beta, mean, var, b):
        s = np.asarray(gamma, np.float64) / np.sqrt(np.asarray(var, np.float64) + BN_EPS)
        A = s
        B = np.asarray(beta, np.float64) + (np.asarray(b, np.float64) - np.asarray(mean, np.float64)) * s
        return A.astype(np.float32), B.astype(np.float32)

    A0, B0 = fold(gamma0, beta0, mean0, var0, b0)
    A1, B1 = fold(gamma1, beta1, mean1, var1, b1)

    shared = {
        "W0": np.asarray(W0, np.float16),
        "W1": np.asarray(W1, np.float16),
        "W2": np.asarray(W2, np.float16),
        "A0": A0.reshape(HID_C, 1), "B0": B0.reshape(HID_C, 1),
        "A1": A1.reshape(HID_C, 1), "B1": B1.reshape(HID_C, 1),
        "b2": np.asarray(b2, np.float32).reshape(1, OUT_C),
    }

    in_maps = []
    for k in range(N_CORES):
        o = plan["orders"][k]
        real = o < NPC
        xk = np.zeros((NPAD, IN_C), np.float16)
        xk[real] = x[k * NPC + o[real]].astype(np.float16)
        in_maps.append({
            "xT": np.ascontiguousarray(xk.T),
            "idxA": plan["idxA"][k],
            "idxB": plan["idxB"][k],
            "dinv_row": plan["dinv_ord"][k].reshape(1, NPAD),
            "dinv_col": np.ascontiguousarray(
                plan["dinv_ord"][k].reshape(TILES, 128).T),
            **shared,
        })

    res = run_bass_kernel_spmd(nc, in_maps, list(range(N_CORES)), trace=TRACE)
    LAST_EXEC_NS = res.exec_time_ns

    out = np.empty((N_NODES, OUT_C), np.float32)
    for k in range(N_CORES):
        o = plan["orders"][k]
        real = o < NPC
        out[k * NPC + o[real]] = res.results[k]["out"][real]
    return out


# revision 29
# speedup vs baseline: 1.0407x; 1.0407x over previous
"""3-layer GCN (GCNConv + BN + ReLU x2 + GCNConv) on 8 Trainium2 NeuronCores.

Strategy (dst-partitioned graph parallelism):
  - Nodes are split across 8 cores (6250 each, padded to 6272 = 49*128).
  - Within a core, nodes are reordered by (n_low_edges, n_high_edges) so each
    128-node output tile has a near-uniform per-node incident-edge count; edge
    slots are padded per-tile to the tile max -> rectangular gather layouts.
  - Per layer: dense transform W on the PE (feature-major, weights stationary),
    scale columns by dinv and cast to fp16, transpose back to node-major rows,
    write a 256B-strided fp16 node-feature table to DRAM, AllGather it,
    then bulk dma_gather (int16 idx; payload C_out*2 bytes, 256B row stride)
    of every edge's source row, and a strided DVE tensor_reduce per output
    tile.  BN+ReLU is a per-feature affine fused into one ACT op per tile.
  - norm(e) = dinv[src]*dinv[dst]: dinv[src] is folded into the table rows,
    dinv[dst] is applied to the aggregated tile.
  - int16 gather indices cover at most 32767 rows, so edges are split by
    source table row at ROW_SPLIT = 5*6272; low/high sides gather from offset
    views of the same table.  The (n_low, n_high) node sort keeps the
    per-tile padding of both sides small.

Everything data-dependent (degrees, permutations, slot layouts, indices) is
precomputed on the host in numpy; the device program is static SPMD (all 8
cores run the same NEFF with different input arrays).
"""
import sys

sys.path.insert(0, "/opt/trn_rl_repo")

import numpy as np

N_NODES = 50000
IN_C, HID_C, OUT_C = 128, 64, 40
BN_EPS = 1e-5
N_CORES = 8
NPC = N_NODES // N_CORES          # 6250
TILES = 49
NPAD = TILES * 128                # 6272
TABLE_ROWS = N_CORES * NPAD       # 50176
OSPLIT = 5 * NPC                  # original-id split (31250)
ROW_SPLIT = 5 * NPAD              # table-row split (31360) < 32768
SLOT_E = 64                       # fp16 elems per table row
SLOT_B = SLOT_E * 2               # bytes per SBUF table rank-slot
RANKS = TABLE_ROWS // 128         # 392 SBUF table slots per partition
BATCH_SLOTS = 32                  # gather slots per dma_gather instruction
N_QUEUES = 4

_COMPILED = {}


# ----------------------------------------------------------------- host plan
def _build_plan(edge_index):
    ei = np.asarray(edge_index)
    loops = np.arange(N_NODES, dtype=np.int64)
    src = np.concatenate([ei[0].astype(np.int64), loops])
    dst = np.concatenate([ei[1].astype(np.int64), loops])
    deg = np.bincount(dst, minlength=N_NODES).astype(np.float64)
    dinv = (1.0 / np.sqrt(deg)).astype(np.float32)

    is_low = src < OSPLIT
    core_of_dst = dst // NPC

    # per-core node stats and ordering (local ids 0..6271; >=6250 are dummies)
    orders = []           # per core: local real/dummy id at each pi position
    nl_ord = np.zeros((N_CORES, NPAD), np.int64)
    nh_ord = np.zeros((N_CORES, NPAD), np.int64)
    pos_of = np.empty(N_NODES, np.int64)  # pi position of each original node
    for k in range(N_CORES):
        m = core_of_dst == k
        dl = dst[m] - k * NPC
        nl = np.bincount(dl[is_low[m]], minlength=NPAD)
        nh = np.bincount(dl[~is_low[m]], minlength=NPAD)
        order = np.lexsort((nh, nl))  # ascending; dummies (0,0) first
        # boustrophedon on the secondary key: within each nl-group alternate
        # nh direction so tiles straddling group boundaries stay homogeneous
        vals = nl[order]
        segs, flip, i = [], False, 0
        while i < len(order):
            j = i
            while j < len(order) and vals[j] == vals[i]:
                j += 1
            segs.append(order[i:j][::-1] if flip else order[i:j])
            flip = not flip
            i = j
        order = np.concatenate(segs)
        # local refinement: within adjacent tile pairs whose nl spread is <=1,
        # re-sort by nh to shave the per-tile nh max
        for _ in range(2):
            for t in range(0, TILES - 1):
                seg = order[t * 128 : (t + 2) * 128]
                a = nl[seg]
                if a.max() - a.min() <= 1:
                    order[t * 128 : (t + 2) * 128] = seg[
                        np.argsort(nh[seg], kind="stable")]
        orders.append(order)
        nl_ord[k] = nl[order]
        nh_ord[k] = nh[order]
        inv = np.empty(NPAD, np.int64)
        inv[order] = np.arange(NPAD)
        pos_of[k * NPC : (k + 1) * NPC] = inv[:NPC]

    row_of = (np.arange(N_NODES) // NPC) * NPAD + pos_of  # table row per node

    # common per-tile slot counts
    kA = nl_ord.reshape(N_CORES, TILES, 128).max(axis=2).max(axis=0)
    kB = nh_ord.reshape(N_CORES, TILES, 128).max(axis=2).max(axis=0)
    baseA = np.concatenate([[0], np.cumsum(kA)[:-1]])
    baseB = np.concatenate([[0], np.cumsum(kB)[:-1]])
    slotsA, slotsB = int(kA.sum()), int(kB.sum())

    Z_LO = 0                       # core 0 dummy row (always zero)
    Z_HI = 7 * NPAD - ROW_SPLIT    # core 7 dummy row, rebased

    def side_vals(k, low):
        m = (core_of_dst == k) & (is_low if low else ~is_low)
        sp = pos_of[dst[m]]  # pi position of dst within its core
        rows = row_of[src[m]]
        o = np.argsort(sp, kind="stable")
        sp, rows = sp[o], rows[o]
        counts = np.bincount(sp, minlength=NPAD)
        starts = np.concatenate([[0], np.cumsum(counts)[:-1]])
        occ = np.arange(len(sp)) - starts[sp]
        t, p = sp // 128, sp % 128
        base = baseA if low else baseB
        nslots = slotsA if low else slotsB
        vals = np.full(nslots * 128, Z_LO if low else Z_HI, np.int64)
        pos = (base[t] + occ) * 128 + p
        vals[pos] = rows if low else rows - ROW_SPLIT
        return vals

    def wrap16(v):
        n = v.shape[0]
        cols = n // 16
        t = v.astype(np.int16).reshape(cols, 16).T
        return np.tile(t, (8, 1))

    idxA = np.stack([wrap16(side_vals(k, True)) for k in range(N_CORES)])
    idxB = np.stack([wrap16(side_vals(k, False)) for k in range(N_CORES)])

    # gather batches: tile-aligned runs with <= BATCH_SLOTS slots
    def make_batches(kvec):
        batches = []  # (tile0, tile1, slot0, nslots)
        t0, s0, acc = 0, 0, 0
        for t in range(TILES):
            if acc + kvec[t] > BATCH_SLOTS and acc > 0:
                batches.append((t0, t, s0, acc))
                t0, s0, acc = t, s0 + acc, 0
            acc += int(kvec[t])
        batches.append((t0, TILES, s0, acc))
        return batches

    batchesA, batchesB = make_batches(kA), make_batches(kB)

    dinv_ord = np.zeros((N_CORES, NPAD), np.float32)
    for k in range(N_CORES):
        o = orders[k]
        real = o < NPC
        dinv_ord[k][real] = dinv[k * NPC + o[real]]

    return dict(
        orders=orders, dinv_ord=dinv_ord, idxA=idxA, idxB=idxB,
        kA=kA, kB=kB, baseA=baseA, baseB=baseB,
        slotsA=slotsA, slotsB=slotsB,
        batchesA=batchesA, batchesB=batchesB,
    )


# ------------------------------------------------------------- device build
def _build_bass(plan):
    import concourse.bacc as bacc
    import concourse.bass as bass
    import concourse.tile as tile
    from concourse import ap_utils, mybir
    from concourse.masks import make_identity

    kA, kB = plan["kA"], plan["kB"]
    baseA, baseB = plan["baseA"], plan["baseB"]
    slotsA, slotsB = plan["slotsA"], plan["slotsB"]
    batchesA, batchesB = plan["batchesA"], plan["batchesB"]
    CA, CB = slotsA * 8, slotsB * 8

    fp16, f32, i16 = mybir.dt.float16, mybir.dt.float32, mybir.dt.int16

    nc = bacc.Bacc(None, target_bir_lowering=False, num_swdge_queues=N_QUEUES)

    xT = nc.declare_dram_parameter("xT", [IN_C, NPAD], fp16, isOutput=False)
    idxA_p = nc.declare_dram_parameter("idxA", [128, CA], i16, isOutput=False)
    idxB_p = nc.declare_dram_parameter("idxB", [128, CB], i16, isOutput=False)
    dinv_row_p = nc.declare_dram_parameter("dinv_row", [1, NPAD], f32, isOutput=False)
    dinv_col_p = nc.declare_dram_parameter("dinv_col", [128, TILES], f32, isOutput=False)
    W_p = [
        nc.declare_dram_parameter("W0", [IN_C, HID_C], fp16, isOutput=False),
        nc.declare_dram_parameter("W1", [HID_C, HID_C], fp16, isOutput=False),
        nc.declare_dram_parameter("W2", [HID_C, OUT_C], fp16, isOutput=False),
    ]
    A_p = [
        nc.declare_dram_parameter("A0", [HID_C, 1], f32, isOutput=False),
        nc.declare_dram_parameter("A1", [HID_C, 1], f32, isOutput=False),
    ]
    B_p = [
        nc.declare_dram_parameter("B0", [HID_C, 1], f32, isOutput=False),
        nc.declare_dram_parameter("B1", [HID_C, 1], f32, isOutput=False),
    ]
    b2_p = nc.declare_dram_parameter("b2", [1, OUT_C], f32, isOutput=False)
    out_p = nc.declare_dram_parameter("out", [NPAD, OUT_C], f32, isOutput=True)

    C_OUT = [HID_C, HID_C, OUT_C]
    C_IN = [IN_C, HID_C, HID_C]

    reg_cache = {}

    def nir(n):
        if n not in reg_cache:
            reg_cache[n] = nc.gpsimd.to_reg(n)
        return reg_cache[n]

    def gather(out_ap, in_ap, idxs_ap, num_idxs, elem_size, queue):
        # SBUF-source gather: idx -> partition idx%128, rank idx//128 at
        # 128B-per-rank free stride within in_ap's base.
        self = nc.gpsimd
        return self.add_instruction(
            mybir.InstDMAGatherAnt(
                name=nc.get_next_instruction_name(),
                ins=[self.lower_ap(in_ap), self.lower_ap(idxs_ap),
                     self.lower_val_access(nir(num_idxs))],
                outs=[self.lower_ap(out_ap)],
                transpose=False,
                num_idxs=num_idxs,
                elem_size=elem_size,
                stride_bytes_256=0,
                gen_mode=0,
                single_packet=False,
                queue_num=queue,
                sbuf_tokens_per_rank=128,
                sbuf_free_dim_per_rank=SLOT_B,
                sbuf_free_dim_pad_per_rank=0,
                sbuf_byte_offset=0,
            )
        )

    with tile.TileContext(nc) as tc:
        with (
            tc.tile_pool(name="const", bufs=1) as constp,
            tc.tile_pool(name="ht", bufs=1) as htp,
            tc.tile_pool(name="work", bufs=3) as work,
            tc.tile_pool(name="gbuf", bufs=1) as gbufp,
            tc.tile_pool(name="zs", bufs=4) as zsp,
            tc.tile_pool(name="psum", bufs=2, space="PSUM") as psum,
            tc.tile_pool(name="psum2", bufs=2, space="PSUM") as psum2,
            tc.tile_pool(name="dram", bufs=1, space="DRAM") as dram,
        ):
            ident = constp.tile([128, 128], fp16)
            make_identity(nc, ident[:])
            idxA_t = constp.tile([128, CA], i16)
            nc.sync.dma_start(out=idxA_t[:], in_=idxA_p[:])
            idxB_t = constp.tile([128, CB], i16)
            nc.sync.dma_start(out=idxB_t[:], in_=idxB_p[:])
            dinv_col = constp.tile([128, TILES], f32)
            nc.sync.dma_start(out=dinv_col[:], in_=dinv_col_p[:])
            W_t = []
            for l in range(3):
                w = constp.tile([C_IN[l], C_OUT[l]], fp16, name=f"W{l}t")
                nc.sync.dma_start(out=w[:], in_=W_p[l][:])
                W_t.append(w)
            AB_t = []
            for l in range(2):
                a = constp.tile([HID_C, 1], f32, name=f"A{l}t")
                nc.sync.dma_start(out=a[:], in_=A_p[l][:])
                b = constp.tile([HID_C, 1], f32, name=f"B{l}t")
                nc.sync.dma_start(out=b[:], in_=B_p[l][:])
                AB_t.append((a, b))
            b2_t = constp.tile([128, OUT_C], f32)
            nc.gpsimd.dma_start(
                out=b2_t[:], in_=b2_p[:].to_broadcast([128, OUT_C]))

            hT = [htp.tile([128, NPAD], fp16, name="hT0"),
                  htp.tile([HID_C, NPAD], fp16, name="hT1")]
            nc.sync.dma_start(out=hT[0][:], in_=xT[:])

            tabin = [dram.tile([128, TILES * SLOT_E], fp16, name=f"tabin{l}")
                     for l in range(3)]
            tabout = [
                dram.tile([N_CORES * 128, TILES * SLOT_E], fp16,
                          addr_space="Shared", name=f"tabout{l}")
                for l in range(3)
            ]
            tab_s = htp.tile([128, RANKS * SLOT_E], fp16, name="tab_s")

            qrr = [0]

            def next_q():
                q = qrr[0] % N_QUEUES
                qrr[0] += 1
                return q

            for l in range(3):
                cin, cout = C_IN[l], C_OUT[l]
                h_cur = hT[l % 2]
                # ---- transform + table build (feature-major) ----
                for c0 in range(0, NPAD, 512):
                    w = min(512, NPAD - c0)
                    pt = psum.tile([cout, 512], f32, tag="pt")
                    nc.tensor.matmul(pt[:, :w], W_t[l][:], h_cur[:cin, c0 : c0 + w],
                                     start=True, stop=True)
                    ts_ = work.tile([cout, 512], fp16, tag="ts")
                    nc.scalar.activation(
                        out=ts_[:, :w], in_=pt[:, :w],
                        func=mybir.ActivationFunctionType.Copy)
                    for j0 in range(0, w, 128):
                        ptr = psum2.tile([128, cout], fp16, tag="ptr")
                        nc.tensor.transpose(
                            ptr[:], ts_[:, j0 : j0 + 128], ident[:cout, :cout])
                        tb = work.tile([128, cout], fp16, tag="tb")
                        # fold dinv[src] into the table row (node-major:
                        # per-partition scale on the ACT engine)
                        nc.scalar.activation(
                            out=tb[:], in_=ptr[:],
                            func=mybir.ActivationFunctionType.Copy,
                            scale=dinv_col[:, (c0 + j0) // 128 :
                                           (c0 + j0) // 128 + 1])
                        t_i = (c0 + j0) // 128
                        nc.sync.dma_start(
                            out=tabin[l][:, t_i * SLOT_E : t_i * SLOT_E + cout],
                            in_=tb[:],
                        )
                # ---- all-gather the fp16 table ----
                nc.gpsimd.collective_compute(
                    "AllGather",
                    mybir.AluOpType.bypass,
                    replica_groups=[list(range(N_CORES))],
                    ins=[tabin[l][:]],
                    outs=[tabout[l][:]],
                )
                # ---- load the gathered table into SBUF ----
                nc.sync.dma_start(
                    out=tab_s[:].rearrange("p (k c) -> p k c", k=N_CORES),
                    in_=tabout[l][:].rearrange("(k p) c -> p k c", p=128),
                )
                # ---- gathers (A/B interleaved) + per-tile reduce/post ----
                gtiles = {}
                sides = (
                    ("A", batchesA, idxA_t, baseA, kA, tab_s[:, :]),
                    ("B", batchesB, idxB_t, baseB, kB,
                     tab_s[:, (ROW_SPLIT // 128) * SLOT_E :]),
                )

                def emit_gather(side_i, bi):
                    side, batches, idx_t, base, kvec, lo = sides[side_i]
                    t0, t1, s0, ns = batches[bi]
                    g = gbufp.tile([128, ns * cout], fp16,
                                   name=f"g{l}{side}{bi}",
                                   tag=f"g{side}{bi % 3}", bufs=2)
                    gather(
                        out_ap=g[:].rearrange("p (s f) -> p s f", f=cout),
                        in_ap=lo,
                        idxs_ap=idx_t[:, s0 * 8 : (s0 + ns) * 8],
                        num_idxs=ns * 128,
                        elem_size=cout,
                        queue=next_q(),
                    )
                    for t in range(t0, t1):
                        off = int(base[t] - s0)
                        gtiles.setdefault(t, {})[side] = (g, off, int(kvec[t]))
                    return t1

                def emit_tile(t):
                    gA, offA, ka = gtiles[t]["A"]
                    gB, offB, kb = gtiles[t]["B"]
                    z = zsp.tile([128, cout], f32, tag="z")
                    if ka > 0:
                        vA = gA[:, offA * cout : (offA + ka) * cout].rearrange(
                            "p (s f) -> p f s", s=ka)
                        nc.vector.tensor_reduce(
                            out=z[:], in_=vA, axis=mybir.AxisListType.X,
                            op=mybir.AluOpType.add)
                    if kb > 0:
                        vB = gB[:, offB * cout : (offB + kb) * cout].rearrange(
                            "p (s f) -> p f s", s=kb)
                        if ka > 0:
                            z2 = zsp.tile([128, cout], f32, tag="z2")
                            nc.vector.tensor_reduce(
                                out=z2[:], in_=vB, axis=mybir.AxisListType.X,
                                op=mybir.AluOpType.add)
                            nc.vector.tensor_add(out=z[:], in0=z[:], in1=z2[:])
                        else:
                            nc.vector.tensor_reduce(
                                out=z[:], in_=vB, axis=mybir.AxisListType.X,
                                op=mybir.AluOpType.add)
                    if l < 2:
                        zsc = zsp.tile([128, cout], fp16, tag="zsc")
                        nc.scalar.activation(
                            out=zsc[:], in_=z[:],
                            func=mybir.ActivationFunctionType.Copy,
                            scale=dinv_col[:, t : t + 1])
                        pz = psum2.tile([cout, 128], fp16, tag="pz")
                        nc.tensor.transpose(pz[:], zsc[:], ident[:])
                        a_t, bb_t = AB_t[l]
                        nc.scalar.activation(
                            out=hT[(l + 1) % 2][:cout, t * 128 : (t + 1) * 128],
                            in_=pz[:],
                            func=mybir.ActivationFunctionType.Relu,
                            bias=bb_t[:],
                            scale=a_t[:],
                        )
                    else:
                        zf = zsp.tile([128, OUT_C], f32, tag="zf")
                        nc.scalar.activation(
                            out=zf[:], in_=z[:],
                            func=mybir.ActivationFunctionType.Copy,
                            scale=dinv_col[:, t : t + 1])
                        nc.vector.tensor_add(out=zf[:], in0=zf[:], in1=b2_t[:])
                        nc.sync.dma_start(
                            out=out_p[t * 128 : (t + 1) * 128, :], in_=zf[:])

                iA = iB = 0
                covA = covB = 0
                done = 0
                while done < TILES:
                    if iA < len(batchesA):
                        covA = emit_gather(0, iA)
                        iA += 1
                    if iB < len(batchesB):
                        covB = emit_gather(1, iB)
                        iB += 1
                    if iA >= len(batchesA):
                        covA = TILES
                    if iB >= len(batchesB):
                        covB = TILES
                    for t in range(done, min(covA, covB)):
                        emit_tile(t)
                    done = max(done, min(covA, covB))

    nc.finalize()
    return nc


# ------------------------------------------------------------------- kernel
TRACE = False
LAST_EXEC_NS = None


def kernel(x, edge_index, W0, b0, gamma0, beta0, mean0, var0,
           W1, b1, gamma1, beta1, mean1, var1, W2, b2):
    global LAST_EXEC_NS
    from concourse.bass_utils import run_bass_kernel_spmd

    x = np.asarray(x, np.float32)
    plan = _build_plan(edge_index)

    key = "k"
    if key not in _COMPILED:
        _COMPILED[key] = _build_bass(plan)
    nc = _COMPILED[key]

    # fold BN (eval mode) into per-feature affines: y = z*A + B
    def fold(gamma, beta, mean, var, b):
        s = np.asarray(gamma, np.float64) / np.sqrt(np.asarray(var, np.float64) + BN_EPS)
        A = s
        B = np.asarray(beta, np.float64) + (np.asarray(b, np.float64) - np.asarray(mean, np.float64)) * s
        return A.astype(np.float32), B.astype(np.float32)

    A0, B0 = fold(gamma0, beta0, mean0, var0, b0)
    A1, B1 = fold(gamma1, beta1, mean1, var1, b1)

    shared = {
        "W0": np.asarray(W0, np.float16),
        "W1": np.asarray(W1, np.float16),
        "W2": np.asarray(W2, np.float16),
        "A0": A0.reshape(HID_C, 1), "B0": B0.reshape(HID_C, 1),
        "A1": A1.reshape(HID_C, 1), "B1": B1.reshape(HID_C, 1),
        "b2": np.asarray(b2, np.float32).reshape(1, OUT_C),
    }

    in_maps = []
    for k in range(N_CORES):
        o = plan["orders"][k]
        real = o < NPC
        xk = np.zeros((NPAD, IN_C), np.float16)
        xk[real] = x[k * NPC + o[real]].astype(np.float16)
        in_maps.append({
            "xT": np.ascontiguousarray(xk.T),
            "idxA": plan["idxA"][k],
            "idxB": plan["idxB"][k],
            "dinv_row": plan["dinv_ord"][k].reshape(1, NPAD),
            "dinv_col": np.ascontiguousarray(
                plan["dinv_ord"][k].reshape(TILES, 128).T),
            **shared,
        })

    res = run_bass_kernel_spmd(nc, in_maps, list(range(N_CORES)), trace=TRACE)
    LAST_EXEC_NS = res.exec_time_ns

    out = np.empty((N_NODES, OUT_C), np.float32)
    for k in range(N_CORES):
        o = plan["orders"][k]
        real = o < NPC
        out[k * NPC + o[real]] = res.results[k]["out"][real]
    return out
